# revision 4
# baseline (speedup 1.0000x reference)
"""GAT layer (nn_GATLayer) Trainium2 Bass kernel, 8-core SPMD — v3.

Same math/sharding as v2 (Y = adj_sl @ G, G = [e*ft | e], row-sharded
destinations, host-marshalled bf16 transposed adj slabs). v3 trims
instruction count and engine columns:
- preproc is 3 ops per m-tile: matmul -> ACT exp (bf16, straight into
  G's e-columns) -> one DVE dual-head multiply reading the PSUM result
  and broadcasting the bf16 e-columns (their rounding cancels in the
  softmax ratio).
- epilogue fuses the PSUM drain with the normalization multiply
  (per n-tile: reciprocal + one multiply), then runs ELU batched over
  all 8 n-tiles (5 ops on [128, 1024]) and writes out with a single
  transpose-pattern DMA.
"""

import numpy as np
import ml_dtypes

import concourse.bass as bass
import concourse.mybir as mybir
import concourse.tile as tile
from concourse import bacc
from concourse.bass_utils import run_bass_kernel_spmd

F32 = mybir.dt.float32
BF16 = mybir.dt.bfloat16
AF = mybir.ActivationFunctionType

N = 8192
IN_DIM = 64
OUT_DIM = 64
HEADS = 2
NCORES = 8
ROWS = N // NCORES           # 1024 destination rows per core
NT = ROWS // 128             # 8 n-tiles per core
MT = N // 128                # 64 m-tiles
C = HEADS * OUT_DIM + HEADS  # 130 columns of G
FT65 = IN_DIM + 1
BF = np.dtype(ml_dtypes.bfloat16)


def _split_heads(ap):
    """[128, 128] contiguous slice -> [128, 2, 64] view."""
    return bass.AP(tensor=ap.tensor, offset=ap.offset,
                   ap=[list(ap.ap[0]), [64, 2], [1, 64]])


def build_program(reps: int = 1, timing: bool = False):
    """timing=True: adjt becomes an Internal DRAM tensor (not transferred
    over the axon tunnel; HW timing is value-independent) and the reps
    run in a tc.For_i hardware loop so the program stays small while the
    kernel body executes `reps` times on device."""
    nc = bacc.Bacc("TRN2", target_bir_lowering=False, debug=False,
                   num_devices=NCORES)

    adjt = nc.dram_tensor("adjt", [NT * 128, MT * 128], BF16,
                          kind="Internal" if timing else "ExternalInput").ap()
    ft65 = nc.dram_tensor("ft65", [FT65, N], F32, kind="ExternalInput").ap()
    wcat = nc.dram_tensor("wcat", [FT65, C], F32, kind="ExternalInput").ap()
    out = nc.dram_tensor("out", [ROWS, HEADS * OUT_DIM], F32,
                         kind="ExternalOutput").ap()
    # out rows n = t*128 + p as [p, t, c] for the single batched store
    out_pt = bass.AP(tensor=out.tensor, offset=0,
                     ap=[[HEADS * OUT_DIM, 128],
                         [128 * HEADS * OUT_DIM, NT],
                         [1, HEADS * OUT_DIM]])

    with tile.TileContext(nc) as tc:
        with tc.tile_pool(name="const", bufs=1) as const, \
             tc.tile_pool(name="gpool", bufs=1) as gpool, \
             tc.tile_pool(name="ppool", bufs=1) as ppool, \
             tc.tile_pool(name="at_p", bufs=3) as at_p, \
             tc.tile_pool(name="ep", bufs=2) as ep, \
             tc.tile_pool(name="ps_g", bufs=2, space="PSUM") as ps_g, \
             tc.tile_pool(name="ps_y", bufs=4, space="PSUM") as ps_y_p:

            ft_sb = const.tile([FT65, N], F32)
            nc.sync.dma_start(out=ft_sb, in_=ft65)
            wc_sb = const.tile([FT65, C], F32)
            nc.sync.dma_start(out=wc_sb, in_=wcat)

            def body(_i=None):
                g = gpool.tile([128, MT, C], BF16, name="g")

                # ---- preproc: matmuls + ACT drains, then batched
                # exp and per-head scaled casts (fewer ops/sem chains)
                pp = ppool.tile([128, MT, C], F32, name="pp")
                for mt in range(MT):
                    psg = ps_g.tile([128, C], F32, name="psg", tag="psg")
                    nc.tensor.matmul(psg, ft_sb[:, mt * 128:(mt + 1) * 128],
                                     wc_sb, start=True, stop=True)
                    nc.scalar.activation(pp[:, mt, :], psg, AF.Copy)
                nc.scalar.activation(g[:, :, 128:130], pp[:, :, 128:130],
                                     AF.Exp)
                for h in range(HEADS):
                    e_rep = bass.AP(tensor=g.tensor,
                                    offset=g.offset + 128 + h,
                                    ap=[list(g.ap[0]), [C, MT], [0, 64]])
                    nc.vector.tensor_mul(g[:, :, h * 64:(h + 1) * 64],
                                         pp[:, :, h * 64:(h + 1) * 64],
                                         e_rep)

                # ---- main: 512 accumulating matmuls + fused epilogue
                obuf = ep.tile([128, NT, 128], F32, name="obuf")
                for t in range(NT):
                    at = at_p.tile([128, MT, 128], BF16, name="at", tag="at")
                    nc.sync.dma_start(
                        out=at.rearrange("p a b -> p (a b)"),
                        in_=adjt[t * 128:(t + 1) * 128, :])
                    ps_y = ps_y_p.tile([128, C], F32, name="ps_y", tag="acc")
                    for mt in range(MT):
                        nc.tensor.matmul(ps_y, at[:, mt, :], g[:, mt, :],
                                         start=(mt == 0), stop=(mt == MT - 1))
                    r2 = ep.tile([128, HEADS], F32, name="r2", tag="r2")
                    nc.vector.reciprocal(r2, ps_y[:, 128:130])
                    r_rep = bass.AP(tensor=r2.tensor, offset=r2.offset,
                                    ap=[list(r2.ap[0]), [1, 2], [0, 64]])
                    nc.vector.tensor_mul(_split_heads(obuf[:, t, :]),
                                         _split_heads(ps_y[:, 0:128]), r_rep)

                # ---- batched ELU over all 8 n-tiles + single store
                of = obuf.rearrange("p a b -> p (a b)")
                mn = ep.tile([128, NT * 128], F32, name="mn")
                nc.vector.tensor_scalar_min(mn, of, 0.0)
                ex = ep.tile([128, NT * 128], F32, name="ex")
                nc.scalar.activation(ex, mn, AF.Exp)
                nc.vector.tensor_scalar_max(of, of, 0.0)
                nc.vector.tensor_add(of, of, ex)
                nc.vector.tensor_scalar_add(of, of, -1.0)
                nc.sync.dma_start(out=out_pt, in_=obuf)

            if timing:
                with tc.For_i(0, reps):
                    body()
            else:
                for _rep in range(reps):
                    body()

    nc.compile()
    return nc


def make_in_maps(adj, features, W_attn, b_attn, W_lin, b_lin):
    adj = np.asarray(adj, dtype=np.float32)
    features = np.asarray(features, dtype=np.float32)
    W_attn = np.asarray(W_attn, dtype=np.float32)
    b_attn = np.asarray(b_attn, dtype=np.float32)
    W_lin = np.asarray(W_lin, dtype=np.float32)
    b_lin = np.asarray(b_lin, dtype=np.float32)

    ft65 = np.concatenate([features.T,
                           np.ones((1, N), np.float32)], axis=0)
    ft65 = np.ascontiguousarray(ft65)
    wcat = np.zeros((FT65, C), np.float32)
    wcat[:IN_DIM, 0:HEADS * OUT_DIM] = W_lin.T
    wcat[:IN_DIM, HEADS * OUT_DIM:] = W_attn[:, IN_DIM:].T
    wcat[IN_DIM, 0:HEADS * OUT_DIM] = b_lin
    wcat[IN_DIM, HEADS * OUT_DIM:] = b_attn

    A = adj.astype(BF)
    idx = np.arange(N)
    A[idx, idx] = (adj[idx, idx] + 1.0).astype(BF)

    in_maps = []
    for c in range(NCORES):
        slab = A[c * ROWS:(c + 1) * ROWS, :]
        adjt = slab.reshape(NT, 128, MT, 128).transpose(0, 3, 2, 1)
        adjt = np.ascontiguousarray(adjt).reshape(NT * 128, MT * 128)
        in_maps.append({"adjt": adjt, "ft65": ft65, "wcat": wcat})
    return in_maps


_CACHED = {}


def _get_program(reps=1, timing=False):
    key = (reps, timing)
    if key not in _CACHED:
        _CACHED[key] = build_program(reps, timing=timing)
    return _CACHED[key]


def run_on_device(in_maps, reps=1, timing=False, **kw):
    nc = _get_program(reps, timing=timing)
    if timing:
        in_maps = [{k: v for k, v in m.items() if k != "adjt"}
                   for m in in_maps]
    res = run_bass_kernel_spmd(nc, in_maps, core_ids=list(range(NCORES)), **kw)
    return res


def kernel(adj, features, W_attn, b_attn, W_lin, b_lin):
    in_maps = make_in_maps(adj, features, W_attn, b_attn, W_lin, b_lin)
    res = run_on_device(in_maps, reps=1)
    return np.concatenate([res.results[c]["out"] for c in range(NCORES)],
                          axis=0)



# revision 6
# speedup vs baseline: 2.2863x; 2.2863x over previous
"""GAT layer (nn_GATLayer) Trainium2 Bass kernel, 8-core SPMD — v4.

Math (exp(s_src) cancels in the softmax):
  out[n,h,:] = ELU( (sum_m A[n,m] e[m,h] ft[m,h,:]) / (sum_m A[n,m] e[m,h]) )
  e[m,h] = exp(s_dst[m,h] + b[h]),  ft = features @ W_lin.T + b_lin
Per core: Y = A_slab @ G with G = [e*ft | e]  ([8192, 130]).

v4 over v3:
- adjacency slab shipped as fp8e4m3 ({0,1,2} exact): halves HBM traffic
  (8.4MB/core) and LDWEIGHTS time (FWL loads 4 fp8/cycle).
- preproc writes G straight from PSUM (exp on ACT, dual-head multiply on
  DVE, per 3-m-tile group) — no pp buffer, no ACT copy drains.
- main loop is mt-outer with all 8 destination-row accumulators packed
  into 3 PSUM banks (130 fp32 at offsets 0/130/260; first matmul in a
  bank start=True, later chains start=False relying on per-element
  has_written overwrite semantics), so a single mt sweep feeds all 8
  n-tiles and the adjacency streams through the PE exactly once.
- all 8 adjacency chunks prefetched up front (contiguous 1MB DMAs).
"""

import numpy as np
import ml_dtypes

import concourse.bass as bass
import concourse.mybir as mybir
import concourse.tile as tile
from concourse import bacc
from concourse.bass_utils import run_bass_kernel_spmd

F32 = mybir.dt.float32
BF16 = mybir.dt.bfloat16
FP8 = mybir.dt.float8e4
AF = mybir.ActivationFunctionType

N = 8192
IN_DIM = 64
OUT_DIM = 64
HEADS = 2
NCORES = 8
ROWS = N // NCORES           # 1024 destination rows per core
NT = ROWS // 128             # 8 n-tiles per core
MT = N // 128                # 64 m-tiles
C = HEADS * OUT_DIM + HEADS  # 130 columns of G
FT65 = IN_DIM + 1
NP_FP8 = np.dtype(ml_dtypes.float8_e4m3)
GRP = 3                      # preproc m-tiles per PSUM bank (3*130*4B<=2KB)
NGRP = (MT + GRP - 1) // GRP
MCHUNK = 8                   # m-tiles per adjacency DMA (1MB each)


def _ap(t, off, dims):
    return bass.AP(tensor=t.tensor, offset=t.offset + off,
                   ap=[list(t.ap[0])] + dims)


def build_program(reps: int = 1, timing: bool = False):
    """timing=True: adjt becomes an Internal DRAM tensor (not transferred
    over the axon tunnel; HW timing is value-independent) and the reps
    run in a tc.For_i hardware loop so the program stays small while the
    kernel body executes `reps` times on device."""
    nc = bacc.Bacc("TRN2", target_bir_lowering=False, debug=False,
                   num_devices=NCORES)

    # adjt host layout: [128 (m within tile), MT, NT, 128 (n within tile)]
    adjt = nc.dram_tensor("adjt", [128, MT * NT * 128], FP8,
                          kind="Internal" if timing else "ExternalInput").ap()
    ftcat = nc.dram_tensor("ftcat", [FT65, N], BF16,
                           kind="ExternalInput").ap()
    wcat = nc.dram_tensor("wcat", [FT65, C], BF16, kind="ExternalInput").ap()
    out = nc.dram_tensor("out", [ROWS, HEADS * OUT_DIM], F32,
                         kind="ExternalOutput").ap()
    # out rows n = t*128 + p as [p, t, c] for the single batched store
    out_pt = bass.AP(tensor=out.tensor, offset=0,
                     ap=[[HEADS * OUT_DIM, 128],
                         [128 * HEADS * OUT_DIM, NT],
                         [1, HEADS * OUT_DIM]])

    with tile.TileContext(nc) as tc:
        with tc.tile_pool(name="const", bufs=1) as const, \
             tc.tile_pool(name="gpool", bufs=1) as gpool, \
             tc.tile_pool(name="atp", bufs=1) as atp, \
             tc.tile_pool(name="ep", bufs=2) as ep, \
             tc.tile_pool(name="ps_g", bufs=4, space="PSUM") as ps_g, \
             tc.tile_pool(name="ps_y", bufs=1, space="PSUM") as ps_y_p:

            ft_sb = const.tile([FT65, N], BF16)
            nc.sync.dma_start(out=ft_sb, in_=ftcat)
            wc_sb = const.tile([FT65, C], BF16)
            nc.sync.dma_start(out=wc_sb, in_=wcat)

            def body(_i=None):
                # ---- adjacency prefetch: 8 contiguous 1MB DMAs
                at = atp.tile([128, MT, NT, 128], FP8, name="at")
                atf = at.rearrange("p a b c -> p (a b c)")
                for ck in range(MT // MCHUNK):
                    w = MCHUNK * NT * 128
                    nc.sync.dma_start(out=atf[:, ck * w:(ck + 1) * w],
                                      in_=adjt[:, ck * w:(ck + 1) * w])

                # ---- preproc: G = [e*ft | e] straight from PSUM,
                # per 3-m-tile group
                g = gpool.tile([128, MT, C], BF16, name="g")
                for k in range(NGRP):
                    m0 = k * GRP
                    nm = min(GRP, MT - m0)
                    psg = ps_g.tile([128, GRP, C], F32, name="psg", tag="psg")
                    for j in range(nm):
                        nc.tensor.matmul(
                            psg[:, j, :],
                            ft_sb[:, (m0 + j) * 128:(m0 + j + 1) * 128],
                            wc_sb, start=True, stop=True)
                    # e-cols: exp(PSUM) -> g[:, m0:m0+nm, 128:130] (bf16)
                    nc.scalar.activation(
                        _ap(g, m0 * C + 128, [[C, nm], [1, HEADS]]),
                        _ap(psg, 128, [[C, nm], [1, HEADS]]),
                        AF.Exp)
                    # ft-cols: psg * e (broadcast bf16 e-cols; their
                    # rounding cancels in the softmax ratio)
                    for h in range(HEADS):
                        nc.vector.tensor_mul(
                            _ap(g, m0 * C + h * 64, [[C, nm], [1, 64]]),
                            _ap(psg, h * 64, [[C, nm], [1, 64]]),
                            _ap(g, m0 * C + 128 + h, [[C, nm], [0, 64]]))

                # ---- main: single mt sweep, 8 accumulators in 3 banks
                ps_y = [ps_y_p.tile([128, GRP, C], F32, name=f"psy{b}",
                                    tag=f"psy{b}") for b in range(3)]
                for mt in range(MT):
                    for t in range(NT):
                        b, j = divmod(t, GRP)
                        nc.tensor.matmul(
                            ps_y[b][:, j, :], at[:, mt, t, :], g[:, mt, :],
                            start=(mt == 0 and j == 0),
                            stop=(mt == MT - 1),
                            skip_group_check=True)

                # ---- epilogue: normalize (batched per bank), ELU, store
                obuf = ep.tile([128, NT, 128], F32, name="obuf")
                r2 = ep.tile([128, NT, HEADS], F32, name="r2")
                for b in range(3):
                    nt = min(GRP, NT - b * GRP)
                    nc.vector.reciprocal(
                        _ap(r2, b * GRP * HEADS, [[HEADS, nt], [1, HEADS]]),
                        _ap(ps_y[b], 128, [[C, nt], [1, HEADS]]))
                    nc.vector.tensor_mul(
                        _ap(obuf, b * GRP * 128,
                            [[128, nt], [64, HEADS], [1, 64]]),
                        _ap(ps_y[b], 0, [[C, nt], [64, HEADS], [1, 64]]),
                        _ap(r2, b * GRP * HEADS,
                            [[HEADS, nt], [1, HEADS], [0, 64]]))

                of = obuf.rearrange("p a b -> p (a b)")
                mn = ep.tile([128, NT * 128], F32, name="mn")
                nc.vector.tensor_scalar_min(mn, of, 0.0)
                ex = ep.tile([128, NT * 128], F32, name="ex")
                nc.scalar.activation(ex, mn, AF.Exp)
                nc.vector.tensor_scalar_max(of, of, 0.0)
                nc.vector.tensor_add(of, of, ex)
                nc.vector.tensor_scalar_add(of, of, -1.0)
                nc.sync.dma_start(out=out_pt, in_=obuf)

            if timing:
                with tc.For_i(0, reps):
                    body()
            else:
                for _rep in range(reps):
                    body()

    nc.compile()
    return nc


def make_in_maps(adj, features, W_attn, b_attn, W_lin, b_lin):
    adj = np.asarray(adj, dtype=np.float32)
    features = np.asarray(features, dtype=np.float32)
    W_attn = np.asarray(W_attn, dtype=np.float32)
    b_attn = np.asarray(b_attn, dtype=np.float32)
    W_lin = np.asarray(W_lin, dtype=np.float32)
    b_lin = np.asarray(b_lin, dtype=np.float32)

    BF = np.dtype(ml_dtypes.bfloat16)
    ftcat = np.concatenate([features.T,
                            np.ones((1, N), np.float32)], axis=0)
    ftcat = np.ascontiguousarray(ftcat).astype(BF)
    wcat = np.zeros((FT65, C), np.float32)
    wcat[:IN_DIM, 0:HEADS * OUT_DIM] = W_lin.T
    wcat[:IN_DIM, HEADS * OUT_DIM:] = W_attn[:, IN_DIM:].T
    wcat[IN_DIM, 0:HEADS * OUT_DIM] = b_lin
    wcat[IN_DIM, HEADS * OUT_DIM:] = b_attn
    wcat = wcat.astype(BF)

    A = adj.astype(NP_FP8)
    idx = np.arange(N)
    A[idx, idx] = (adj[idx, idx] + 1.0).astype(NP_FP8)

    in_maps = []
    for c in range(NCORES):
        slab = A[c * ROWS:(c + 1) * ROWS, :]       # [1024 n, 8192 m]
        # -> [128 (m in tile), MT, NT, 128 (n in tile)]
        adjt = slab.reshape(NT, 128, MT, 128).transpose(3, 2, 0, 1)
        adjt = np.ascontiguousarray(adjt).reshape(128, MT * NT * 128)
        in_maps.append({"adjt": adjt, "ftcat": ftcat, "wcat": wcat})
    return in_maps


_CACHED = {}


def _get_program(reps=1, timing=False):
    key = (reps, timing)
    if key not in _CACHED:
        _CACHED[key] = build_program(reps, timing=timing)
    return _CACHED[key]


def run_on_device(in_maps, reps=1, timing=False, **kw):
    nc = _get_program(reps, timing=timing)
    if timing:
        in_maps = [{k: v for k, v in m.items() if k != "adjt"}
                   for m in in_maps]
    res = run_bass_kernel_spmd(nc, in_maps, core_ids=list(range(NCORES)), **kw)
    return res


def kernel(adj, features, W_attn, b_attn, W_lin, b_lin):
    in_maps = make_in_maps(adj, features, W_attn, b_attn, W_lin, b_lin)
    res = run_on_device(in_maps, reps=1)
    return np.concatenate([res.results[c]["out"] for c in range(NCORES)],
                          axis=0)
